# revision 19
# baseline (speedup 1.0000x reference)
"""Bass/Tile Trainium2 kernel for nn_Attention_14620068676191.

Math (per batch element b, data-parallel over 8 cores):
    q = x @ Wq^T ; k = x @ Wk^T
    scores = q @ k^T / sqrt(D)  ==  x @ (Wq^T Wk) @ x^T / sqrt(D)
    out = softmax(tanh(scores), axis=-1) @ x

We fold the two projections into M = Wq^T @ Wk (computed host-side), so the
per-core work is
    y  = x @ M                      [S, D]
    S^T = x @ y^T  (t on partitions, s on free dim)
    A^T = exp(tanh(S^T / sqrt(D)))  (no max-subtraction needed: tanh bounds
                                     the scores to [-1, 1])
    O_ext = A @ [x | 1]             (ones column gives the softmax
                                     denominator Z in the same matmul)
    out = O_ext[:, :D] / Z
All matmuls run in bf16 (fp32 PSUM accumulation); inputs are converted to
bf16 host-side. Measured end-to-end absmax relative error vs the fp32
reference is ~3e-3.

Scheduling notes (from perfetto/NTFF traces):
  * The PE matmul stream is issue-bound at ~215 ns per 512-wide bf16 matmul
    (2.4 GHz); the stream itself is within ~3% of optimal, so the wins are in
    the DMA-paced start and the serial tail.
  * Input DMAs are chunked (x: 2+2 then 4-row-block quads, M: 2-block
    chunks) and ordered so the x-transposes and the first y q-block start
    while the input tail is still in flight.  M is row-permuted host-side so
    each SBUF partition's 8 rows are DRAM-contiguous (16 KB descriptors:
    software-DGE descriptor generation, not transfer bandwidth, is the
    input-load bottleneck).
  * The y projection is emitted q-block-outer with e-pairs accumulating in
    2-bank PSUM tiles borrowed from the (then idle) PV pool — the first
    score matmuls unblock as early as possible and the PE->DVE handoff
    count halves.
  * The softmax denominator comes from a 2-wide ones-column matmul per PV
    block (costs ~28 ns/tile of PE issue); in the last accumulation step it
    is issued first so the reciprocal overlaps the final output matmuls.
  * The final normalize runs scalar-engine half + DVE half into separate
    tiles (a shared tile serializes on a WAW dep) so the last store issues
    as early as the cross-engine semaphore latency allows.
"""

from contextlib import ExitStack

import ml_dtypes
import numpy as np

import concourse.bass as bass
import concourse.tile as tile
from concourse import bacc, mybir
from concourse.bass import ds, ts
from concourse.bass_utils import run_bass_kernel_spmd
from concourse.masks import make_identity

S, B, D = 2048, 8, 1024
P = 128
NS, ND = S // P, D // P  # 16, 8
NB = 512                 # matmul moving-operand block (one PSUM bank fp32)
NQ = S // NB             # 4 s-blocks
DX = D + 2               # x rows padded: col D = 1.0 (softmax denom), col D+1 = 0
F32, BF16 = mybir.dt.float32, mybir.dt.bfloat16
AF = mybir.ActivationFunctionType
ISCALE = float(D) ** -0.5

N_CORES = 8


def _emit(ctx: ExitStack, tc: tile.TileContext, x_d, m_d, o_d):
    nc = tc.nc

    consts = ctx.enter_context(tc.tile_pool(name="consts", bufs=1))
    pool_xbf = ctx.enter_context(tc.tile_pool(name="xbf", bufs=1))
    pool_xt = ctx.enter_context(tc.tile_pool(name="xt", bufs=1))
    pool_m = ctx.enter_context(tc.tile_pool(name="mw", bufs=1))
    pool_yt = ctx.enter_context(tc.tile_pool(name="yt", bufs=1))
    # 16KB/partition scratch slots for the A^T tiles
    pool_big = ctx.enter_context(tc.tile_pool(name="big", bufs=2))
    pool_osb = ctx.enter_context(tc.tile_pool(name="osb", bufs=3))
    pool_rz = ctx.enter_context(tc.tile_pool(name="rz", bufs=3))
    psum_mm = ctx.enter_context(tc.tile_pool(name="pmm", bufs=4, space="PSUM"))
    psum_pv = ctx.enter_context(tc.tile_pool(name="ppv", bufs=2, space="PSUM"))

    ident = consts.tile([P, P], BF16)

    x_bf = pool_xbf.tile([P, NS, DX], BF16)  # x_bf[p, i, d] = x[i*P+p, d]
    xT = pool_xt.tile([P, ND, S], BF16)      # xT[p, j, s]  = x[s, j*P+p]
    m_bf = pool_m.tile([P, ND, D], BF16)     # m_bf[p, j, e] = M[j*P+p, e]
    yT = pool_yt.tile([P, ND, S], BF16)      # yT[p, j, s]  = y[s, j*P+p]

    # ---- chunked loads: x pairs 0-1, M (host-transposed, 1 desc/partition),
    # then the rest of x in quads.  DMA issues go first so the software-DGE
    # descriptor generation overlaps the make_identity setup.
    x_r = x_d.rearrange("(i p) d -> p i d", p=P)
    m_r = m_d.rearrange("(p j) e -> p j e", p=P)
    for i in range(0, 4, 2):
        nc.gpsimd.dma_start(x_bf[:, i : i + 2, 0:D], x_r[:, i : i + 2])
    make_identity(nc, ident)
    for j in range(0, ND, 2):
        nc.gpsimd.dma_start(m_bf[:, j : j + 2], m_r[:, j : j + 2])
    for i in range(4, NS, 4):
        nc.gpsimd.dma_start(x_bf[:, i : i + 4, 0:D], x_r[:, i : i + 4])
    nc.gpsimd.memset(x_bf[:, :, D : D + 1], 1.0)
    nc.gpsimd.memset(x_bf[:, :, D + 1 : DX], 0.0)

    # ---- PE-transpose x into xT (one PSUM bank holds all 8 j per i) --------
    def transpose_chunk(i):
        tp = psum_mm.tile([P, ND, P], BF16, tag="mm")
        for jj in range(ND):
            nc.tensor.transpose(tp[:, jj], x_bf[:, i, ds(jj * P, P)], ident)
        nc.vector.tensor_copy(xT[:, :, ts(i, P)], tp)

    # ---- y^T[e, s] = sum_d M[d, e] * x[s, d] for one q block ----------------
    # e-pairs accumulate in a 2-bank PSUM tile from the (then idle) PV pool:
    # halves the PE->DVE handoff count vs one tile per e.
    def y_block(q):
        for e in range(0, ND, 2):
            ps = psum_pv.tile([P, 2, NB], F32, tag="po")
            for dch in range(ND):
                st_, sp_ = dch == 0, dch == ND - 1
                nc.tensor.matmul(
                    ps[:, 0], m_bf[:, dch, ts(e, P)], xT[:, dch, ts(q, NB)],
                    start=st_, stop=sp_,
                )
                nc.tensor.matmul(
                    ps[:, 1], m_bf[:, dch, ts(e + 1, P)], xT[:, dch, ts(q, NB)],
                    start=st_, stop=sp_,
                )
            nc.vector.tensor_copy(yT[:, e : e + 2, ts(q, NB)], ps)

    for i in range(4):
        transpose_chunk(i)
    y_block(0)
    for i in range(4, NS):
        transpose_chunk(i)
    for q in range(1, NQ):
        y_block(q)

    # ---- per s-block: scores^T -> tanh -> exp -> PV -> normalize -> store ---
    for q in range(NQ):
        at = pool_big.tile([P, NS, NB], BF16, tag="big")
        for t_i in range(NS):
            ps = psum_mm.tile([P, NB], F32, tag="mm")
            for e in range(ND):
                nc.tensor.matmul(
                    ps,
                    xT[:, e, ts(t_i, P)],
                    yT[:, e, ts(q, NB)],
                    start=(e == 0),
                    stop=(e == ND - 1),
                )
            nc.scalar.activation(at[:, t_i, :], ps, AF.Tanh, scale=ISCALE)
            nc.scalar.activation(at[:, t_i, :], at[:, t_i, :], AF.Exp)
        for ss in range(NB // P):
            st = q * (NB // P) + ss
            po = psum_pv.tile([P, 2, NB], F32, tag="po")
            pz = psum_mm.tile([P, 2], F32, tag="mm")
            for t_i in range(NS):
                lw = at[:, t_i, ts(ss, P)]
                first, last = t_i == 0, t_i == NS - 1
                if last:
                    # denominator first so the reciprocal can start while the
                    # two output matmuls finish
                    nc.tensor.matmul(pz, lw, x_bf[:, t_i, D:DX], start=first, stop=last)
                    nc.tensor.matmul(po[:, 0], lw, x_bf[:, t_i, 0:NB], start=first, stop=last)
                    nc.tensor.matmul(po[:, 1], lw, x_bf[:, t_i, NB:D], start=first, stop=last)
                else:
                    nc.tensor.matmul(po[:, 0], lw, x_bf[:, t_i, 0:NB], start=first, stop=last)
                    nc.tensor.matmul(po[:, 1], lw, x_bf[:, t_i, NB:D], start=first, stop=last)
                    nc.tensor.matmul(pz, lw, x_bf[:, t_i, D:DX], start=first, stop=last)
            r = pool_rz.tile([P, 1], F32, tag="rz")
            nc.vector.reciprocal(r, pz[:, 0:1])
            # normalize the two halves on different engines in parallel
            # (separate tiles: a shared tile would serialize on a WAW dep)
            osb0 = pool_osb.tile([P, NB], F32, tag="osb0")
            osb1 = pool_osb.tile([P, NB], F32, tag="osb1")
            nc.scalar.mul(osb0, po[:, 0], r)
            nc.vector.tensor_scalar_mul(osb1, po[:, 1], r)
            nc.gpsimd.dma_start(o_d[ts(st, P), 0:NB], osb0)
            nc.gpsimd.dma_start(o_d[ts(st, P), NB:D], osb1)


def build_program() -> bass.Bass:
    nc = bacc.Bacc("TRN2", target_bir_lowering=False, debug=False)
    x_d = nc.declare_dram_parameter("x", [S, D], BF16, isOutput=False)
    m_d = nc.declare_dram_parameter("m", [D, D], BF16, isOutput=False)
    o_d = nc.declare_dram_parameter("out", [S, D], F32, isOutput=True)
    with tile.TileContext(nc) as tc:
        with ExitStack() as ctx:
            _emit(ctx, tc, x_d.ap(), m_d.ap(), o_d.ap())
    nc.compile()
    return nc


_CACHE: dict = {}


def _get_program() -> bass.Bass:
    if "nc" not in _CACHE:
        _CACHE["nc"] = build_program()
    return _CACHE["nc"]


def run(x, Wq, Wk, trace: bool = False):
    """Run on 8 NeuronCores (batch-parallel). Returns (out, BassKernelResults)."""
    x = np.asarray(x, dtype=np.float32)
    wq = np.asarray(Wq, dtype=np.float32)
    wk = np.asarray(Wk, dtype=np.float32)
    m_full = (wq.T @ wk).astype(ml_dtypes.bfloat16)
    # rows reordered (j*128+p) -> (p*8+j) so each SBUF partition's 8 rows are
    # contiguous in DRAM (single 16KB DMA descriptor per partition)
    m = np.ascontiguousarray(
        m_full.reshape(ND, P, D).transpose(1, 0, 2).reshape(D, D)
    )
    nc = _get_program()
    in_maps = [
        {
            "x": np.ascontiguousarray(x[:, b, :].astype(ml_dtypes.bfloat16)),
            "m": m,
        }
        for b in range(N_CORES)
    ]
    res = run_bass_kernel_spmd(nc, in_maps, list(range(N_CORES)), trace=trace)
    out = np.stack([res.results[b]["out"] for b in range(N_CORES)], axis=1)
    return out, res


def kernel(x, Wq, Wk):
    out, _ = run(x, Wq, Wk)
    return out


# revision 20
# speedup vs baseline: 1.0085x; 1.0085x over previous
"""Bass/Tile Trainium2 kernel for nn_Attention_14620068676191.

Math (per batch element b, data-parallel over 8 cores):
    q = x @ Wq^T ; k = x @ Wk^T
    scores = q @ k^T / sqrt(D)  ==  x @ (Wq^T Wk) @ x^T / sqrt(D)
    out = softmax(tanh(scores), axis=-1) @ x

We fold the two projections into M = Wq^T @ Wk (computed host-side), so the
per-core work is
    y  = x @ M                      [S, D]
    S^T = x @ y^T  (t on partitions, s on free dim)
    A^T = exp(tanh(S^T / sqrt(D)))  (no max-subtraction needed: tanh bounds
                                     the scores to [-1, 1])
    O_ext = A @ [x | 1]             (ones column gives the softmax
                                     denominator Z in the same matmul)
    out = O_ext[:, :D] / Z
All matmuls run in bf16 (fp32 PSUM accumulation); inputs are converted to
bf16 host-side. Measured end-to-end absmax relative error vs the fp32
reference is ~3e-3.

Scheduling notes (from perfetto/NTFF traces):
  * The PE matmul stream is issue-bound at ~215 ns per 512-wide bf16 matmul
    (2.4 GHz); the stream itself is within ~3% of optimal, so the wins are in
    the DMA-paced start and the serial tail.
  * Input DMAs are chunked (x: 2+2 then 4-row-block quads, M: 2-block
    chunks) and ordered so the x-transposes and the first y q-block start
    while the input tail is still in flight.  M is row-permuted host-side so
    each SBUF partition's 8 rows are DRAM-contiguous (16 KB descriptors:
    software-DGE descriptor generation, not transfer bandwidth, is the
    input-load bottleneck).
  * The y projection is emitted q-block-outer with e-pairs accumulating in
    2-bank PSUM tiles borrowed from the (then idle) PV pool — the first
    score matmuls unblock as early as possible and the PE->DVE handoff
    count halves.
  * The softmax denominator comes from a 2-wide ones-column matmul per PV
    block (costs ~28 ns/tile of PE issue); in the last accumulation step it
    is issued first so the reciprocal overlaps the final output matmuls.
  * The final normalize runs scalar-engine half + DVE half into separate
    tiles (a shared tile serializes on a WAW dep) so the last store issues
    as early as the cross-engine semaphore latency allows.
"""

from contextlib import ExitStack

import ml_dtypes
import numpy as np

import concourse.bass as bass
import concourse.tile as tile
from concourse import bacc, mybir
from concourse.bass import ds, ts
from concourse.bass_utils import run_bass_kernel_spmd
from concourse.masks import make_identity

S, B, D = 2048, 8, 1024
P = 128
NS, ND = S // P, D // P  # 16, 8
NB = 512                 # matmul moving-operand block (one PSUM bank fp32)
NQ = S // NB             # 4 s-blocks
DX = D + 2               # x rows padded: col D = 1.0 (softmax denom), col D+1 = 0
F32, BF16 = mybir.dt.float32, mybir.dt.bfloat16
AF = mybir.ActivationFunctionType
ISCALE = float(D) ** -0.5

N_CORES = 8


def _emit(ctx: ExitStack, tc: tile.TileContext, x_d, m_d, o_d):
    nc = tc.nc

    consts = ctx.enter_context(tc.tile_pool(name="consts", bufs=1))
    pool_xbf = ctx.enter_context(tc.tile_pool(name="xbf", bufs=1))
    pool_xt = ctx.enter_context(tc.tile_pool(name="xt", bufs=1))
    pool_m = ctx.enter_context(tc.tile_pool(name="mw", bufs=1))
    pool_yt = ctx.enter_context(tc.tile_pool(name="yt", bufs=1))
    # 16KB/partition scratch slots for the A^T tiles
    pool_big = ctx.enter_context(tc.tile_pool(name="big", bufs=2))
    pool_osb = ctx.enter_context(tc.tile_pool(name="osb", bufs=3))
    pool_rz = ctx.enter_context(tc.tile_pool(name="rz", bufs=3))
    psum_mm = ctx.enter_context(tc.tile_pool(name="pmm", bufs=4, space="PSUM"))
    psum_pv = ctx.enter_context(tc.tile_pool(name="ppv", bufs=2, space="PSUM"))

    ident = consts.tile([P, P], BF16)

    x_bf = pool_xbf.tile([P, NS, DX], BF16)  # x_bf[p, i, d] = x[i*P+p, d]
    xT = pool_xt.tile([P, ND, S], BF16)      # xT[p, j, s]  = x[s, j*P+p]
    m_bf = pool_m.tile([P, ND, D], BF16)     # m_bf[p, j, e] = M[j*P+p, e]
    yT = pool_yt.tile([P, ND, S], BF16)      # yT[p, j, s]  = y[s, j*P+p]

    # ---- chunked loads: x pairs 0-1, M (host-transposed, 1 desc/partition),
    # then the rest of x in quads.  DMA issues go first so the software-DGE
    # descriptor generation overlaps the make_identity setup.
    x_r = x_d.rearrange("(i p) d -> p i d", p=P)
    m_r = m_d.rearrange("(p j) e -> p j e", p=P)
    for i in range(0, 4, 2):
        nc.gpsimd.dma_start(x_bf[:, i : i + 2, 0:D], x_r[:, i : i + 2])
    make_identity(nc, ident)
    for j in range(0, ND, 2):
        nc.gpsimd.dma_start(m_bf[:, j : j + 2], m_r[:, j : j + 2])
    for i in range(4, NS, 4):
        nc.gpsimd.dma_start(x_bf[:, i : i + 4, 0:D], x_r[:, i : i + 4])
    nc.gpsimd.memset(x_bf[:, :, D : D + 1], 1.0)
    nc.gpsimd.memset(x_bf[:, :, D + 1 : DX], 0.0)

    # ---- PE-transpose x into xT (4 j-blocks per PSUM tile) -----------------
    def transpose_chunk(i):
        for j0 in range(0, ND, 4):
            tp = psum_mm.tile([P, 4, P], BF16, tag="mm")
            for jj in range(4):
                nc.tensor.transpose(tp[:, jj], x_bf[:, i, ds((j0 + jj) * P, P)], ident)
            nc.vector.tensor_copy(xT[:, j0 : j0 + 4, ts(i, P)], tp)

    # ---- y^T[e, s] = sum_d M[d, e] * x[s, d] for one q block ----------------
    # e-pairs accumulate in a 2-bank PSUM tile from the (then idle) PV pool:
    # halves the PE->DVE handoff count vs one tile per e.
    def y_block(q):
        for e in range(0, ND, 2):
            ps = psum_pv.tile([P, 2, NB], F32, tag="po")
            for dch in range(ND):
                st_, sp_ = dch == 0, dch == ND - 1
                nc.tensor.matmul(
                    ps[:, 0], m_bf[:, dch, ts(e, P)], xT[:, dch, ts(q, NB)],
                    start=st_, stop=sp_,
                )
                nc.tensor.matmul(
                    ps[:, 1], m_bf[:, dch, ts(e + 1, P)], xT[:, dch, ts(q, NB)],
                    start=st_, stop=sp_,
                )
            nc.vector.tensor_copy(yT[:, e : e + 2, ts(q, NB)], ps)

    for i in range(4):
        transpose_chunk(i)
    y_block(0)
    for i in range(4, NS):
        transpose_chunk(i)
    for q in range(1, NQ):
        y_block(q)

    # ---- per s-block: scores^T -> tanh -> exp -> PV -> normalize -> store ---
    for q in range(NQ):
        at = pool_big.tile([P, NS, NB], BF16, tag="big")
        for t_i in range(NS):
            ps = psum_mm.tile([P, NB], F32, tag="mm")
            for e in range(ND):
                nc.tensor.matmul(
                    ps,
                    xT[:, e, ts(t_i, P)],
                    yT[:, e, ts(q, NB)],
                    start=(e == 0),
                    stop=(e == ND - 1),
                )
            nc.scalar.activation(at[:, t_i, :], ps, AF.Tanh, scale=ISCALE)
            nc.scalar.activation(at[:, t_i, :], at[:, t_i, :], AF.Exp)
        for ss in range(NB // P):
            st = q * (NB // P) + ss
            po = psum_pv.tile([P, 2, NB], F32, tag="po")
            pz = psum_mm.tile([P, 2], F32, tag="mm")
            for t_i in range(NS):
                lw = at[:, t_i, ts(ss, P)]
                first, last = t_i == 0, t_i == NS - 1
                if last:
                    # denominator first so the reciprocal can start while the
                    # two output matmuls finish
                    nc.tensor.matmul(pz, lw, x_bf[:, t_i, D:DX], start=first, stop=last)
                    nc.tensor.matmul(po[:, 0], lw, x_bf[:, t_i, 0:NB], start=first, stop=last)
                    nc.tensor.matmul(po[:, 1], lw, x_bf[:, t_i, NB:D], start=first, stop=last)
                else:
                    nc.tensor.matmul(po[:, 0], lw, x_bf[:, t_i, 0:NB], start=first, stop=last)
                    nc.tensor.matmul(po[:, 1], lw, x_bf[:, t_i, NB:D], start=first, stop=last)
                    nc.tensor.matmul(pz, lw, x_bf[:, t_i, D:DX], start=first, stop=last)
            r = pool_rz.tile([P, 1], F32, tag="rz")
            nc.vector.reciprocal(r, pz[:, 0:1])
            # normalize the two halves on different engines in parallel
            # (separate tiles: a shared tile would serialize on a WAW dep)
            osb0 = pool_osb.tile([P, NB], F32, tag="osb0")
            osb1 = pool_osb.tile([P, NB], F32, tag="osb1")
            nc.scalar.mul(osb0, po[:, 0], r)
            nc.vector.tensor_scalar_mul(osb1, po[:, 1], r)
            nc.gpsimd.dma_start(o_d[ts(st, P), 0:NB], osb0)
            nc.gpsimd.dma_start(o_d[ts(st, P), NB:D], osb1)


def build_program() -> bass.Bass:
    nc = bacc.Bacc("TRN2", target_bir_lowering=False, debug=False)
    x_d = nc.declare_dram_parameter("x", [S, D], BF16, isOutput=False)
    m_d = nc.declare_dram_parameter("m", [D, D], BF16, isOutput=False)
    o_d = nc.declare_dram_parameter("out", [S, D], F32, isOutput=True)
    with tile.TileContext(nc) as tc:
        with ExitStack() as ctx:
            _emit(ctx, tc, x_d.ap(), m_d.ap(), o_d.ap())
    nc.compile()
    return nc


_CACHE: dict = {}


def _get_program() -> bass.Bass:
    if "nc" not in _CACHE:
        _CACHE["nc"] = build_program()
    return _CACHE["nc"]


def run(x, Wq, Wk, trace: bool = False):
    """Run on 8 NeuronCores (batch-parallel). Returns (out, BassKernelResults)."""
    x = np.asarray(x, dtype=np.float32)
    wq = np.asarray(Wq, dtype=np.float32)
    wk = np.asarray(Wk, dtype=np.float32)
    m_full = (wq.T @ wk).astype(ml_dtypes.bfloat16)
    # rows reordered (j*128+p) -> (p*8+j) so each SBUF partition's 8 rows are
    # contiguous in DRAM (single 16KB DMA descriptor per partition)
    m = np.ascontiguousarray(
        m_full.reshape(ND, P, D).transpose(1, 0, 2).reshape(D, D)
    )
    nc = _get_program()
    in_maps = [
        {
            "x": np.ascontiguousarray(x[:, b, :].astype(ml_dtypes.bfloat16)),
            "m": m,
        }
        for b in range(N_CORES)
    ]
    res = run_bass_kernel_spmd(nc, in_maps, list(range(N_CORES)), trace=trace)
    out = np.stack([res.results[b]["out"] for b in range(N_CORES)], axis=1)
    return out, res


def kernel(x, Wq, Wk):
    out, _ = run(x, Wq, Wk)
    return out
